# revision 2
# baseline (speedup 1.0000x reference)
"""DOSLoss Trainium2 kernel.

Full inputs in, scalar loss out. Internally: pure data-parallel shard of the
batch axis across 8 NeuronCores. Each core streams its shard of cls_score
([8,512,1000]) and n ([8,512,256]) through a Bass/Tile kernel that computes
the two per-(b,k) contractions:

    expsum[b,k] = sum_c exp(cls_score[b,k,c])      (ACT engine, fused accum)
    d2[b,k]     = sum_d (deep_feats[b,d]-n[b,k,d])^2  (DVE sub + fused sq-reduce)

Device layout: partition p = k % 128, SBUF column col = b*4 + j (j = k//128).
The O(B*K) scalar tail (log, sqrt, masked softmax over ragged lengths, target
gather, final sums) runs on host in float64, and the 8 per-core partials are
reduced on host.
"""

import os

import numpy as np

B, KMAX, D, C = 64, 512, 256, 1000
N_CORES = 8
BS = B // N_CORES  # samples per core
P = 128
J = KMAX // P  # k-chunks per sample
NCOL = BS * J  # 32 result columns per core

_CACHE = {}
LAST_RESULTS = None  # BassKernelResults of the most recent device run


def _build_nc():
    import concourse.bacc as bacc
    import concourse.mybir as mybir
    import concourse.tile as tile

    f32 = mybir.dt.float32
    nc = bacc.Bacc("TRN2", target_bir_lowering=False, debug=False)

    cls_t = nc.dram_tensor("cls", [BS, KMAX, C], f32, kind="ExternalInput")
    n_t = nc.dram_tensor("nn", [BS, KMAX, D], f32, kind="ExternalInput")
    fb_t = nc.dram_tensor("fb", [P, BS * D], f32, kind="ExternalInput")
    out_t = nc.dram_tensor("out", [P, 2 * NCOL], f32, kind="ExternalOutput")

    # k = j*128 + p  ->  partition p, free dims (j, inner)
    cls_r = cls_t.ap().rearrange("b (j p) c -> b p j c", p=P)
    n_r = n_t.ap().rearrange("b (j p) d -> b p j d", p=P)

    with tile.TileContext(nc) as tc:
        with (
            tc.tile_pool(name="cls_pool", bufs=3) as cls_pool,
            tc.tile_pool(name="n_pool", bufs=3) as n_pool,
            tc.tile_pool(name="scr_pool", bufs=2) as scr_pool,
            tc.tile_pool(name="acc", bufs=1) as acc,
        ):
            fb = acc.tile([P, BS * D], f32)
            nc.sync.dma_start(out=fb, in_=fb_t.ap())
            res = acc.tile([P, 2 * NCOL], f32)  # cols [0,32): expsum, [32,64): d2

            for b in range(BS):
                ctile = cls_pool.tile([P, J, C], f32, tag="cls")
                nc.sync.dma_start(out=ctile, in_=cls_r[b])
                ntile = n_pool.tile([P, J, D], f32, tag="nn")
                nc.sync.dma_start(out=ntile, in_=n_r[b])
                for j in range(J):
                    col = b * J + j
                    scr = scr_pool.tile([P, C], f32, tag="scr")
                    nc.scalar.activation(
                        out=scr,
                        in_=ctile[:, j, :],
                        func=mybir.ActivationFunctionType.Exp,
                        accum_out=res[:, col : col + 1],
                    )
                    # NOTE: tensor_tensor_reduce reliably faults the exec unit on
                    # this HW/axon stack — use sub + mul + reduce_sum instead.
                    diff = scr_pool.tile([P, D], f32, tag="diff")
                    nc.vector.tensor_sub(diff, ntile[:, j, :], fb[:, b * D : (b + 1) * D])
                    sq = scr_pool.tile([P, D], f32, tag="sq")
                    nc.vector.tensor_mul(sq, diff, diff)
                    nc.vector.reduce_sum(
                        out=res[:, NCOL + col : NCOL + col + 1],
                        in_=sq,
                        axis=mybir.AxisListType.X,
                    )

            nc.sync.dma_start(out=out_t.ap(), in_=res)

    nc.compile()
    return nc


def _get_nc():
    if "nc" not in _CACHE:
        _CACHE["nc"] = _build_nc()
    return _CACHE["nc"]


def _run_device(in_maps):
    global LAST_RESULTS
    from concourse import bass_utils

    nc = _get_nc()
    trace = bool(int(os.environ.get("DOS_TRACE", "0")))
    try:
        results = bass_utils.run_bass_kernel_spmd(
            nc, in_maps, core_ids=list(range(N_CORES)), trace=trace
        )
    except Exception:
        # transient NRT hiccups (e.g. NRT_EXEC_UNIT_UNRECOVERABLE) resolve on retry
        results = bass_utils.run_bass_kernel_spmd(
            nc, in_maps, core_ids=list(range(N_CORES)), trace=trace
        )
    LAST_RESULTS = results
    return [r["out"] for r in results.results]


def kernel(deep_feats, n, w, cls_score, target, lengths):
    deep_feats = np.ascontiguousarray(np.asarray(deep_feats, dtype=np.float32))
    n = np.ascontiguousarray(np.asarray(n, dtype=np.float32))
    w = np.asarray(w, dtype=np.float32)
    cls_score = np.ascontiguousarray(np.asarray(cls_score, dtype=np.float32))
    target = np.asarray(target).astype(np.int64)
    lengths = np.asarray(lengths).astype(np.int64)

    in_maps = []
    for c in range(N_CORES):
        lo, hi = c * BS, (c + 1) * BS
        fb = np.ascontiguousarray(
            np.broadcast_to(deep_feats[lo:hi][None, :, :], (P, BS, D)).reshape(P, BS * D)
        )
        in_maps.append(
            {
                "cls": np.ascontiguousarray(cls_score[lo:hi]),
                "nn": np.ascontiguousarray(n[lo:hi]),
                "fb": fb,
            }
        )

    outs = _run_device(in_maps)

    # [P, 2*NCOL] per core -> [B, KMAX] expsum / d2, with k = j*128 + p
    expsum = np.empty((B, KMAX), dtype=np.float64)
    d2 = np.empty((B, KMAX), dtype=np.float64)
    for c in range(N_CORES):
        o = outs[c].astype(np.float64)  # [128, 64]
        es = o[:, :NCOL].reshape(P, BS, J).transpose(1, 2, 0).reshape(BS, KMAX)
        dd = o[:, NCOL:].reshape(P, BS, J).transpose(1, 2, 0).reshape(BS, KMAX)
        expsum[c * BS : (c + 1) * BS] = es
        d2[c * BS : (c + 1) * BS] = dd

    # host tail in float64
    lse = np.log(expsum)  # [B, KMAX]
    dist = np.sqrt(np.maximum(d2, 0.0))  # [B, KMAX]
    mask = (np.arange(KMAX)[None, :] < lengths[:, None]).astype(np.float64)
    s = -w.astype(np.float64) * dist
    f_loss = float(np.sum(s * mask))

    smax = np.max(np.where(mask > 0, s, -np.inf), axis=1, keepdims=True)
    e = np.exp(s - smax) * mask
    rho = e / np.sum(e, axis=1, keepdims=True)

    cls_at = cls_score[np.arange(B)[:, None], np.arange(KMAX)[None, :], target[:, None]]
    ce = lse - cls_at.astype(np.float64)
    g_loss = float(np.sum(rho * ce))

    return np.float32(f_loss + g_loss)


# revision 3
# speedup vs baseline: 1.0051x; 1.0051x over previous
"""DOSLoss Trainium2 kernel.

Full inputs in, scalar loss out. Internally: pure data-parallel shard of the
batch axis across 8 NeuronCores. Each core streams its shard of cls_score
([8,512,1000]) and n ([8,512,256]) through a Bass/Tile kernel that computes
the two per-(b,k) contractions:

    expsum[b,k] = sum_c exp(cls_score[b,k,c])      (ACT engine, fused accum)
    d2[b,k]     = sum_d (deep_feats[b,d]-n[b,k,d])^2  (DVE sub + fused sq-reduce)

Device layout: partition p = k % 128, SBUF column col = b*4 + j (j = k//128).
The O(B*K) scalar tail (log, sqrt, masked softmax over ragged lengths, target
gather, final sums) runs on host in float64, and the 8 per-core partials are
reduced on host.
"""

import os

import numpy as np

B, KMAX, D, C = 64, 512, 256, 1000
N_CORES = 8
BS = B // N_CORES  # samples per core
P = 128
J = KMAX // P  # k-chunks per sample
NCOL = BS * J  # 32 result columns per core

_CACHE = {}
LAST_RESULTS = None  # BassKernelResults of the most recent device run


def _build_nc():
    import concourse.bacc as bacc
    import concourse.mybir as mybir
    import concourse.tile as tile

    f32 = mybir.dt.float32
    nc = bacc.Bacc("TRN2", target_bir_lowering=False, debug=False)

    cls_t = nc.dram_tensor("cls", [BS, KMAX, C], f32, kind="ExternalInput")
    n_t = nc.dram_tensor("nn", [BS, KMAX, D], f32, kind="ExternalInput")
    fb_t = nc.dram_tensor("fb", [P, BS * D], f32, kind="ExternalInput")
    out_t = nc.dram_tensor("out", [P, 2 * NCOL], f32, kind="ExternalOutput")

    # k = j*128 + p  ->  partition p, free dims (j, inner)
    cls_r = cls_t.ap().rearrange("b (j p) c -> b p j c", p=P)
    n_r = n_t.ap().rearrange("b (j p) d -> b p j d", p=P)

    with tile.TileContext(nc) as tc:
        with (
            tc.tile_pool(name="cls_pool", bufs=5) as cls_pool,
            tc.tile_pool(name="n_pool", bufs=5) as n_pool,
            tc.tile_pool(name="scr_pool", bufs=4) as scr_pool,
            tc.tile_pool(name="acc", bufs=1) as acc,
        ):
            fb = acc.tile([P, BS * D], f32)
            nc.sync.dma_start(out=fb, in_=fb_t.ap())
            res = acc.tile([P, 2 * NCOL], f32)  # cols [0,32): expsum, [32,64): d2

            for b in range(BS):
                ctile = cls_pool.tile([P, J, C], f32, tag="cls")
                nc.sync.dma_start(out=ctile, in_=cls_r[b])
                ntile = n_pool.tile([P, J, D], f32, tag="nn")
                nc.sync.dma_start(out=ntile, in_=n_r[b])
                for j in range(J):
                    col = b * J + j
                    scr = scr_pool.tile([P, C], f32, tag="scr")
                    nc.scalar.activation(
                        out=scr,
                        in_=ctile[:, j, :],
                        func=mybir.ActivationFunctionType.Exp,
                        accum_out=res[:, col : col + 1],
                    )
                # NOTE: tensor_tensor_reduce reliably faults the exec unit on
                # this HW/axon stack — use sub + mul + reduce_sum instead.
                # One wide op per sample (all J chunks at once) minimizes DVE
                # instruction count and per-op DRAIN overhead.
                diff4 = scr_pool.tile([P, J, D], f32, tag="diff4")
                nc.vector.tensor_sub(
                    diff4,
                    ntile,
                    fb[:, b * D : (b + 1) * D]
                    .rearrange("p (o d) -> p o d", o=1)
                    .broadcast_to((P, J, D)),
                )
                sq4 = scr_pool.tile([P, J, D], f32, tag="sq4")
                nc.vector.tensor_mul(sq4, diff4, diff4)
                nc.vector.reduce_sum(
                    out=res[:, NCOL + b * J : NCOL + (b + 1) * J],
                    in_=sq4,
                    axis=mybir.AxisListType.X,
                )

            nc.sync.dma_start(out=out_t.ap(), in_=res)

    nc.compile()
    return nc


def _get_nc():
    if "nc" not in _CACHE:
        _CACHE["nc"] = _build_nc()
    return _CACHE["nc"]


def _run_device(in_maps):
    global LAST_RESULTS
    from concourse import bass_utils

    nc = _get_nc()
    trace = bool(int(os.environ.get("DOS_TRACE", "0")))
    try:
        results = bass_utils.run_bass_kernel_spmd(
            nc, in_maps, core_ids=list(range(N_CORES)), trace=trace
        )
    except Exception:
        # transient NRT hiccups (e.g. NRT_EXEC_UNIT_UNRECOVERABLE) resolve on retry
        results = bass_utils.run_bass_kernel_spmd(
            nc, in_maps, core_ids=list(range(N_CORES)), trace=trace
        )
    LAST_RESULTS = results
    return [r["out"] for r in results.results]


def kernel(deep_feats, n, w, cls_score, target, lengths):
    deep_feats = np.ascontiguousarray(np.asarray(deep_feats, dtype=np.float32))
    n = np.ascontiguousarray(np.asarray(n, dtype=np.float32))
    w = np.asarray(w, dtype=np.float32)
    cls_score = np.ascontiguousarray(np.asarray(cls_score, dtype=np.float32))
    target = np.asarray(target).astype(np.int64)
    lengths = np.asarray(lengths).astype(np.int64)

    in_maps = []
    for c in range(N_CORES):
        lo, hi = c * BS, (c + 1) * BS
        fb = np.ascontiguousarray(
            np.broadcast_to(deep_feats[lo:hi][None, :, :], (P, BS, D)).reshape(P, BS * D)
        )
        in_maps.append(
            {
                "cls": np.ascontiguousarray(cls_score[lo:hi]),
                "nn": np.ascontiguousarray(n[lo:hi]),
                "fb": fb,
            }
        )

    outs = _run_device(in_maps)

    # [P, 2*NCOL] per core -> [B, KMAX] expsum / d2, with k = j*128 + p
    expsum = np.empty((B, KMAX), dtype=np.float64)
    d2 = np.empty((B, KMAX), dtype=np.float64)
    for c in range(N_CORES):
        o = outs[c].astype(np.float64)  # [128, 64]
        es = o[:, :NCOL].reshape(P, BS, J).transpose(1, 2, 0).reshape(BS, KMAX)
        dd = o[:, NCOL:].reshape(P, BS, J).transpose(1, 2, 0).reshape(BS, KMAX)
        expsum[c * BS : (c + 1) * BS] = es
        d2[c * BS : (c + 1) * BS] = dd

    # host tail in float64
    lse = np.log(expsum)  # [B, KMAX]
    dist = np.sqrt(np.maximum(d2, 0.0))  # [B, KMAX]
    mask = (np.arange(KMAX)[None, :] < lengths[:, None]).astype(np.float64)
    s = -w.astype(np.float64) * dist
    f_loss = float(np.sum(s * mask))

    smax = np.max(np.where(mask > 0, s, -np.inf), axis=1, keepdims=True)
    e = np.exp(s - smax) * mask
    rho = e / np.sum(e, axis=1, keepdims=True)

    cls_at = cls_score[np.arange(B)[:, None], np.arange(KMAX)[None, :], target[:, None]]
    ce = lse - cls_at.astype(np.float64)
    g_loss = float(np.sum(rho * ce))

    return np.float32(f_loss + g_loss)
